# revision 16
# baseline (speedup 1.0000x reference)
"""Trainium2 Bass kernel for nn_KernelShiftedPrediction (v5).

For each pixel, over 9 shifts (x,y) in {-1,0,1}^2 (priority order:
(0,0) first, then row-major), pick the shifted `predicted` value
minimizing |target - candidate| with strict first-occurrence
tie-breaking; out-of-bounds shifts never win (60000 fp16 padding).

Strategy (vs the v3 baseline's fp32 3-op DVE update at ~670us):
 - fp16 end to end (validated rel err ~1.4e-2 < 2e-2 gate); host
   converts inputs, HBM traffic halves.
 - signed difference d = c - t is both the argmin KEY (|d|) and the
   PAYLOAD (c = t + d): no candidate tensor is tracked.
 - custom DVE op MERGEMIN_ANT: bd = |d| < |bd| ? d : bd. One 1x DVE
   instruction replaces abs + is_lt + min + copy_predicated. Strict <
   keeps the earlier (higher-priority) shift on ties, matching the
   reference's first-occurrence rule.
 - engine split measured to avoid the GPSIMD<->DVE shared-SBUF-port
   contention (GPSIMD tensor ops would slow concurrent DVE ops ~3x):
   GPSIMD only memsets pads. PE computes 4 of the 9 differences via
   paired matmuls (I@c + (-I)@t) into PSUM; MERGEMIN consumes those
   directly from PSUM (fp32 - no ACT pass). DVE does the other 5
   subtracts (all on 4B-aligned views), the 8-merge chain, and the
   final reconstruction add.

Sharding: batch dim B=8 -> 8 NeuronCores; per core 10 images of
[512,512] as 4 row-chunks of 128 partitions side by side in the free
dim. Vertical shifts via three row-shifted DMA views; horizontal
shifts are free-dim offsets into column-padded (SEG=514) view tiles.
A separate unpadded center view keeps the (0,0) subtract 4B-aligned.
"""
import sys

sys.path.insert(0, "/opt/trn_rl_repo")

import numpy as np

S, B, H, W = 10, 8, 512, 512
CH = 128          # chunk rows (partitions)
NCH = H // CH     # 4 segments per image, side by side
SEG = W + 2       # per-segment width in padded view tiles
FREE_T = NCH * W      # 2048
FREE_P = NCH * SEG    # 2056
PADVAL = 60000.0  # finite fp16 pad; |pad - t| never wins
MMW = 512         # matmul free width (one PSUM bank)

# full priority order, (0,0) first; strict < in the merge keeps earlier
# shifts on ties. PE computes the starred ones; DVE the rest.
SHIFTS = [(0, 0), (-1, -1), (-1, 0), (-1, 1), (0, -1),
          (0, 1), (1, -1), (1, 0), (1, 1)]
PE_SUBS = {(-1, 0), (1, -1), (1, 0), (1, 1)}

_CACHE = {}


def _register_mergemin():
    """Register the MERGEMIN_ANT custom DVE op into concourse.dve_ops.

    out[k] = in0[k] if |in0[k]| < |in1[k]| else in1[k]
    Self-contained (the shared repo does not ship this op); idempotent.
    """
    import concourse.dve_ops as dve_ops
    from concourse.dve_ops import DveOp
    from concourse.dve_spec import Spec, Src0, Src1, Zero, lower, maxx, select
    from concourse.dve_uop import DveOpSpec

    for op in dve_ops.OPS:
        if op.name == "MERGEMIN_ANT":
            return op

    a_abs = maxx(Src0, Zero - Src0)
    b_abs = maxx(Src1, Zero - Src1)
    spec = Spec(
        body=select(a_abs < b_abs, Src0, Src1),
        reference=lambda in0, in1, s0, s1, imm2: np.where(
            np.abs(in0) < np.abs(in1), in0, in1
        ).astype(np.float32),
    )

    name = "MERGEMIN_ANT"
    row = dve_ops._CUSTOM_DVE_ROW_BASE + len(dve_ops.OPS)
    dve_ops._SUB_OPCODE_FOR_NAME[name] = row
    assert row < 0x20

    shas = {}
    for ver in ("v3", "v4"):
        try:
            uops = lower(spec, ver=ver)
            shas[ver] = DveOpSpec(
                name=name, opcode=row, uops=uops, rd1_en=True
            ).sha(ver)
        except Exception:
            pass

    op = DveOp(name, spec, subdim=False, uops_sha=shas)
    dve_ops.OPS.append(op)
    return op


def _build_nc():
    import concourse.bacc as bacc
    import concourse.mybir as mybir
    from concourse.tile import TileContext

    F16 = mybir.dt.float16
    F32 = mybir.dt.float32
    OP = mybir.AluOpType
    MERGEMIN = _register_mergemin()

    nc = bacc.Bacc("TRN2", target_bir_lowering=False, debug=False, num_devices=B)
    pred = nc.declare_dram_parameter("pred", [S, H, W], F16, isOutput=False)
    targ = nc.declare_dram_parameter("targ", [S, H, W], F16, isOutput=False)
    eye2 = nc.declare_dram_parameter("eye2", [128, 256], F16, isOutput=False)
    out = nc.declare_dram_parameter("out", [S, H, W], F16, isOutput=True)

    with TileContext(nc) as tc:
        with (
            tc.tile_pool(name="cst", bufs=1) as cst,
            tc.tile_pool(name="io", bufs=3) as io,
            tc.tile_pool(name="dp", bufs=2) as dp,
            tc.tile_pool(name="ps", bufs=2, space="PSUM") as psp,
        ):
            eye = cst.tile([128, 256], F16)
            nc.sync.dma_start(out=eye[:, :], in_=eye2[:, :])
            W_I = eye[:, 0:128]     # identity
            W_N = eye[:, 128:256]   # -identity

            for s in range(S):
                T = io.tile([CH, FREE_T], F16, tag="T")
                Cp = io.tile([CH, FREE_T], F16, tag="Cp")   # unpadded center
                PU = io.tile([CH, FREE_P], F16, tag="PU")
                PC = io.tile([CH, FREE_P], F16, tag="PC")
                PD = io.tile([CH, FREE_P], F16, tag="PD")

                # column pads: both edges of every segment
                for V in (PU, PC, PD):
                    ap = V[:, :].rearrange("p (g e) -> p g e", g=NCH)
                    nc.gpsimd.memset(ap[:, :, 0:SEG:SEG - 1], PADVAL)

                # row-edge pads (set before DMAs partially overwrite)
                nc.gpsimd.memset(PU[0:32, 0:SEG], PADVAL)
                nc.gpsimd.memset(PD[96:CH, (NCH - 1) * SEG : NCH * SEG], PADVAL)

                # one DMA per tile (3D APs) -- Sync-engine issue time per
                # dma_start is ~0.9us, so fewer/larger transfers matter.
                # pr[p, g, w] = pred[s, g*128 + p, w]
                pr = pred[s, :, :].rearrange("(g p) w -> p g w", g=NCH)
                tg = targ[s, :, :].rearrange("(g p) w -> p g w", g=NCH)
                tview = T[:, :].rearrange("p (g w) -> p g w", g=NCH)
                cview = Cp[:, :].rearrange("p (g w) -> p g w", g=NCH)
                pcv = PC[:, :].rearrange("p (g e) -> p g e", g=NCH)
                puv = PU[:, :].rearrange("p (g e) -> p g e", g=NCH)
                pdv = PD[:, :].rearrange("p (g e) -> p g e", g=NCH)
                nc.sync.dma_start(out=tview, in_=tg)
                nc.sync.dma_start(out=cview, in_=pr)
                nc.sync.dma_start(out=pcv[:, :, 1 : 1 + W], in_=pr)
                # PU[p, g] = pred row g*128+p-1
                nc.sync.dma_start(
                    out=puv[1:CH, :, 1 : 1 + W], in_=pr[0 : CH - 1, :, :]
                )
                nc.sync.dma_start(
                    out=puv[0:1, 1:NCH, 1 : 1 + W], in_=pr[CH - 1 : CH, 0 : NCH - 1, :]
                )
                # PD[p, g] = pred row g*128+p+1
                nc.sync.dma_start(
                    out=pdv[0 : CH - 1, :, 1 : 1 + W], in_=pr[1:CH, :, :]
                )
                nc.sync.dma_start(
                    out=pdv[CH - 1 : CH, 0 : NCH - 1, 1 : 1 + W], in_=pr[0:1, 1:NCH, :]
                )

                VX = {-1: PU, 0: PC, 1: PD}

                def cand(x, y):
                    v = VX[x][:, :].rearrange("p (g w) -> p g w", g=NCH)
                    return v[:, :, 1 + y : 1 + y + W]

                def cand_seg(x, y, g):
                    return VX[x][:, g * SEG + 1 + y : g * SEG + 1 + y + W]

                def g3(t):
                    return t[:, :].rearrange("p (g w) -> p g w", g=NCH)

                # PE difference blocks first so their PSUM tiles are ready
                # when the merge chain reaches them
                srcs = {}
                for x, y in SHIFTS[1:]:
                    if (x, y) not in PE_SUBS:
                        continue
                    ps = psp.tile([CH, FREE_T], F32, tag="ps")
                    for g in range(NCH):
                        nc.tensor.matmul(
                            ps[:, g * W : (g + 1) * W], W_I,
                            cand_seg(x, y, g), start=True, stop=False,
                        )
                        nc.tensor.matmul(
                            ps[:, g * W : (g + 1) * W], W_N,
                            T[:, g * W : (g + 1) * W], start=False, stop=True,
                        )
                    srcs[(x, y)] = ps[:, :]

                # (0,0) seeds the chain; remaining DVE subtracts
                bd = dp.tile([CH, FREE_T], F16, tag="bd")
                nc.vector.tensor_tensor(bd[:, :], Cp[:, :], T[:, :], OP.subtract)
                for i, (x, y) in enumerate(SHIFTS[1:]):
                    if (x, y) in PE_SUBS:
                        continue
                    d = dp.tile([CH, FREE_T], F16, tag=f"d{i}")
                    nc.vector.tensor_tensor(g3(d), cand(x, y), g3(T), OP.subtract)
                    srcs[(x, y)] = d[:, :]

                for x, y in SHIFTS[1:]:
                    nc.vector._custom_dve(
                        MERGEMIN, out=bd[:, :], in0=srcs[(x, y)], in1=bd[:, :]
                    )

                # reconstruct winning candidate: c = t + d
                o = dp.tile([CH, FREE_T], F16, tag="o")
                nc.vector.tensor_tensor(o[:, :], T[:, :], bd[:, :], OP.add)

                og = out[s, :, :].rearrange("(g p) w -> p g w", g=NCH)
                nc.sync.dma_start(
                    out=og, in_=o[:, :].rearrange("p (g w) -> p g w", g=NCH)
                )
    nc.finalize()
    return nc


def _get_nc():
    if "nc" not in _CACHE:
        _CACHE["nc"] = _build_nc()
    return _CACHE["nc"]


def kernel(predicted, target, mask=None, _want_results_obj=False, _trace=False):
    """predicted [S,B,H,W], target [B,S,H,W] -> [S,B,H,W] (mask unused)."""
    from concourse.bass_utils import run_bass_kernel_spmd

    nc = _get_nc()
    eye = np.eye(128, dtype=np.float16)
    eye2 = np.concatenate([eye, -eye], axis=1)
    in_maps = []
    for b in range(B):
        in_maps.append(
            {
                "pred": np.ascontiguousarray(predicted[:, b]).astype(np.float16),
                "targ": np.ascontiguousarray(target[b]).astype(np.float16),
                "eye2": eye2,
            }
        )
    res = run_bass_kernel_spmd(nc, in_maps, list(range(B)), trace=_trace)
    outp = np.stack(
        [res.results[b]["out"].astype(np.float32) for b in range(B)], axis=1
    )
    if _want_results_obj:
        return outp, res
    return outp


# revision 18
# speedup vs baseline: 1.9258x; 1.9258x over previous
"""Trainium2 Bass kernel for nn_KernelShiftedPrediction (v5).

For each pixel, over 9 shifts (x,y) in {-1,0,1}^2 (priority order:
(0,0) first, then row-major), pick the shifted `predicted` value
minimizing |target - candidate| with strict first-occurrence
tie-breaking; out-of-bounds shifts never win (60000 fp16 padding).

Strategy (vs the v3 baseline's fp32 3-op DVE update at ~670us):
 - fp16 end to end (validated rel err ~1.4e-2 < 2e-2 gate); host
   converts inputs, HBM traffic halves.
 - signed difference d = c - t is both the argmin KEY (|d|) and the
   PAYLOAD (c = t + d): no candidate tensor is tracked.
 - custom DVE op MERGEMIN_ANT: bd = |d| < |bd| ? d : bd. One 1x DVE
   instruction replaces abs + is_lt + min + copy_predicated. Strict <
   keeps the earlier (higher-priority) shift on ties, matching the
   reference's first-occurrence rule.
 - engine split measured to avoid the GPSIMD<->DVE shared-SBUF-port
   contention (GPSIMD tensor ops would slow concurrent DVE ops ~3x):
   GPSIMD only memsets pads. PE computes 4 of the 9 differences via
   paired matmuls (I@c + (-I)@t) into PSUM; MERGEMIN consumes those
   directly from PSUM (fp32 - no ACT pass). DVE does the other 5
   subtracts (all on 4B-aligned views), the 8-merge chain, and the
   final reconstruction add.

Sharding: batch dim B=8 -> 8 NeuronCores; per core 10 images of
[512,512] as 4 row-chunks of 128 partitions side by side in the free
dim. Vertical shifts via three row-shifted DMA views; horizontal
shifts are free-dim offsets into column-padded (SEG=514) view tiles.
A separate unpadded center view keeps the (0,0) subtract 4B-aligned.
"""
import sys

sys.path.insert(0, "/opt/trn_rl_repo")

import numpy as np

S, B, H, W = 10, 8, 512, 512
CH = 128          # chunk rows (partitions)
NCH = H // CH     # 4 segments per image, side by side
SEG = W + 2       # per-segment width in padded view tiles
FREE_T = NCH * W      # 2048
FREE_P = NCH * SEG    # 2056
PADVAL = 60000.0  # finite fp16 pad; |pad - t| never wins
MMW = 512         # matmul free width (one PSUM bank)

# full priority order, (0,0) first; strict < in the merge keeps earlier
# shifts on ties. PE computes the starred ones; DVE the rest.
SHIFTS = [(0, 0), (-1, -1), (-1, 0), (-1, 1), (0, -1),
          (0, 1), (1, -1), (1, 0), (1, 1)]
PE_SUBS = {(-1, 0), (1, -1), (1, 0), (1, 1)}

_CACHE = {}


def _register_mergemin():
    """Register the MERGEMIN_ANT custom DVE op into concourse.dve_ops.

    out[k] = in0[k] if |in0[k]| < |in1[k]| else in1[k]
    Self-contained (the shared repo does not ship this op); idempotent.
    """
    import concourse.dve_ops as dve_ops
    from concourse.dve_ops import DveOp
    from concourse.dve_spec import Spec, Src0, Src1, Zero, lower, maxx, select
    from concourse.dve_uop import DveOpSpec

    for op in dve_ops.OPS:
        if op.name == "MERGEMIN_ANT":
            return op

    a_abs = maxx(Src0, Zero - Src0)
    b_abs = maxx(Src1, Zero - Src1)
    spec = Spec(
        body=select(a_abs < b_abs, Src0, Src1),
        reference=lambda in0, in1, s0, s1, imm2: np.where(
            np.abs(in0) < np.abs(in1), in0, in1
        ).astype(np.float32),
    )

    name = "MERGEMIN_ANT"
    row = dve_ops._CUSTOM_DVE_ROW_BASE + len(dve_ops.OPS)
    dve_ops._SUB_OPCODE_FOR_NAME[name] = row
    assert row < 0x20

    shas = {}
    for ver in ("v3", "v4"):
        try:
            uops = lower(spec, ver=ver)
            shas[ver] = DveOpSpec(
                name=name, opcode=row, uops=uops, rd1_en=True
            ).sha(ver)
        except Exception:
            pass

    op = DveOp(name, spec, subdim=False, uops_sha=shas)
    dve_ops.OPS.append(op)
    return op


def _build_nc():
    import concourse.bacc as bacc
    import concourse.mybir as mybir
    from concourse.tile import TileContext

    F16 = mybir.dt.float16
    F32 = mybir.dt.float32
    OP = mybir.AluOpType
    MERGEMIN = _register_mergemin()

    nc = bacc.Bacc("TRN2", target_bir_lowering=False, debug=False, num_devices=B)
    pred = nc.declare_dram_parameter("pred", [S, H, W], F16, isOutput=False)
    targ = nc.declare_dram_parameter("targ", [S, H, W], F16, isOutput=False)
    eye2 = nc.declare_dram_parameter("eye2", [128, 256], F16, isOutput=False)
    out = nc.declare_dram_parameter("out", [S, H, W], F16, isOutput=True)

    with TileContext(nc) as tc:
        with (
            tc.tile_pool(name="cst", bufs=1) as cst,
            tc.tile_pool(name="io", bufs=3) as io,
            tc.tile_pool(name="dp", bufs=2) as dp,
            tc.tile_pool(name="ps", bufs=2, space="PSUM") as psp,
        ):
            eye = cst.tile([128, 256], F16)
            nc.sync.dma_start(out=eye[:, :], in_=eye2[:, :])
            W_I = eye[:, 0:128]     # identity
            W_N = eye[:, 128:256]   # -identity

            for s in range(S):
                T = io.tile([CH, FREE_T], F16, tag="T")
                Cp = io.tile([CH, FREE_T], F16, tag="Cp")   # unpadded center
                PU = io.tile([CH, FREE_P], F16, tag="PU")
                PC = io.tile([CH, FREE_P], F16, tag="PC")
                PD = io.tile([CH, FREE_P], F16, tag="PD")

                # column pads: both edges of every segment
                for V in (PU, PC, PD):
                    ap = V[:, :].rearrange("p (g e) -> p g e", g=NCH)
                    nc.gpsimd.memset(ap[:, :, 0:SEG:SEG - 1], PADVAL)

                # row-edge pads (set before DMAs partially overwrite)
                nc.gpsimd.memset(PU[0:32, 0:SEG], PADVAL)
                nc.gpsimd.memset(PD[96:CH, (NCH - 1) * SEG : NCH * SEG], PADVAL)

                for g in range(NCH):
                    r0 = g * CH
                    cs = g * SEG + 1
                    nc.sync.dma_start(
                        out=T[:, g * W : (g + 1) * W], in_=targ[s, r0 : r0 + CH, :]
                    )
                    nc.sync.dma_start(
                        out=Cp[:, g * W : (g + 1) * W], in_=pred[s, r0 : r0 + CH, :]
                    )
                    nc.sync.dma_start(
                        out=PC[:, cs : cs + W], in_=pred[s, r0 : r0 + CH, :]
                    )
                    if g == 0:
                        nc.sync.dma_start(
                            out=PU[1:CH, cs : cs + W], in_=pred[s, 0 : CH - 1, :]
                        )
                    else:
                        nc.sync.dma_start(
                            out=PU[:, cs : cs + W],
                            in_=pred[s, r0 - 1 : r0 + CH - 1, :],
                        )
                    if g == NCH - 1:
                        nc.sync.dma_start(
                            out=PD[0 : CH - 1, cs : cs + W], in_=pred[s, r0 + 1 : H, :]
                        )
                    else:
                        nc.sync.dma_start(
                            out=PD[:, cs : cs + W],
                            in_=pred[s, r0 + 1 : r0 + CH + 1, :],
                        )

                VX = {-1: PU, 0: PC, 1: PD}

                def cand(x, y):
                    v = VX[x][:, :].rearrange("p (g w) -> p g w", g=NCH)
                    return v[:, :, 1 + y : 1 + y + W]

                def cand_seg(x, y, g):
                    return VX[x][:, g * SEG + 1 + y : g * SEG + 1 + y + W]

                def g3(t):
                    return t[:, :].rearrange("p (g w) -> p g w", g=NCH)

                # PE difference blocks first so their PSUM tiles are ready
                # when the merge chain reaches them
                srcs = {}
                for x, y in SHIFTS[1:]:
                    if (x, y) not in PE_SUBS:
                        continue
                    ps = psp.tile([CH, FREE_T], F32, tag="ps")
                    for g in range(NCH):
                        nc.tensor.matmul(
                            ps[:, g * W : (g + 1) * W], W_I,
                            cand_seg(x, y, g), start=True, stop=False,
                        )
                        nc.tensor.matmul(
                            ps[:, g * W : (g + 1) * W], W_N,
                            T[:, g * W : (g + 1) * W], start=False, stop=True,
                        )
                    srcs[(x, y)] = ps[:, :]

                # (0,0) seeds the chain; remaining DVE subtracts
                bd = dp.tile([CH, FREE_T], F16, tag="bd")
                nc.vector.tensor_tensor(bd[:, :], Cp[:, :], T[:, :], OP.subtract)
                for i, (x, y) in enumerate(SHIFTS[1:]):
                    if (x, y) in PE_SUBS:
                        continue
                    d = dp.tile([CH, FREE_T], F16, tag=f"d{i}")
                    nc.vector.tensor_tensor(g3(d), cand(x, y), g3(T), OP.subtract)
                    srcs[(x, y)] = d[:, :]

                for x, y in SHIFTS[1:]:
                    nc.vector._custom_dve(
                        MERGEMIN, out=bd[:, :], in0=srcs[(x, y)], in1=bd[:, :]
                    )

                # reconstruct winning candidate: c = t + d
                o = dp.tile([CH, FREE_T], F16, tag="o")
                nc.vector.tensor_tensor(o[:, :], T[:, :], bd[:, :], OP.add)

                for g in range(NCH):
                    nc.sync.dma_start(
                        out=out[s, g * CH : (g + 1) * CH, :],
                        in_=o[:, g * W : (g + 1) * W],
                    )
    nc.finalize()
    return nc


def _get_nc():
    if "nc" not in _CACHE:
        _CACHE["nc"] = _build_nc()
    return _CACHE["nc"]


def kernel(predicted, target, mask=None, _want_results_obj=False, _trace=False):
    """predicted [S,B,H,W], target [B,S,H,W] -> [S,B,H,W] (mask unused)."""
    from concourse.bass_utils import run_bass_kernel_spmd

    nc = _get_nc()
    eye = np.eye(128, dtype=np.float16)
    eye2 = np.concatenate([eye, -eye], axis=1)
    in_maps = []
    for b in range(B):
        in_maps.append(
            {
                "pred": np.ascontiguousarray(predicted[:, b]).astype(np.float16),
                "targ": np.ascontiguousarray(target[b]).astype(np.float16),
                "eye2": eye2,
            }
        )
    res = run_bass_kernel_spmd(nc, in_maps, list(range(B)), trace=_trace)
    outp = np.stack(
        [res.results[b]["out"].astype(np.float32) for b in range(B)], axis=1
    )
    if _want_results_obj:
        return outp, res
    return outp
